# revision 2
# baseline (speedup 1.0000x reference)
"""Trainium2 Bass kernel v3 for nn_Attention (B=4, C=512, T=8, H=14, W=14).

Math: tokens tok[b, n, c], n = t*H*W + h*W + w; q = k = v; head hd takes
channels c = d*8 + hd.  Per (b, hd): S = q q^T / 64, P = softmax_rows(S),
out = P q, stored as outT[d, n] with outT[:, n] *= 1/rowsum[n].

v3 vs v2: v2 processed head PAIRS with S psum bufs=1 per head — every
block's matmuls waited on the previous block's exp read (serial
MM->ACT->MM chain, ~2 cross-engine hops/block; measured ~35us/pair of
pure stall).  v3 processes heads SEQUENTIALLY so one head gets 6 psum
banks = [128,1536] x bufs=2, and row-tiles BLOCK PAIRS of the same head
(even blocks in PE rows 0-63 from a duplicated qT copy, odd blocks in
rows 64-127): PE runs 2 blocks ahead of ACT, the exp stream never waits.

 - Host supplies "q" = qT [D, N] per head and "qn" = token-major
   [128, 13*64] per head; no PE transposes on device.
 - OUT(h0, h1) is col-tiled across the head pair (A into PSUM partitions
   0-63, B into 64-127, one bank) and emitted in small chunks between
   S(h2)/S(h3) blocks; OUT(h2, h3) drains at the end.
 - PSUM (8 banks): S [128,1536] x2 bufs = 6, tail bank 1, out accum 1.
 - outT / R are [128, N] pair tiles; output stored bf16.

Engine budget per core: ACT ~94us (13 exps + tail per head, incl
accum_out), PE ~45us, DVE ~15us; per-rep target ~112us incl fill/drain.
"""

import sys

if "/opt/trn_rl_repo" not in sys.path:
    sys.path.insert(0, "/opt/trn_rl_repo")

import ml_dtypes
import numpy as np

import concourse.bass as bass
import concourse.mybir as mybir
import concourse.tile as tile
from concourse import bacc, bass_utils

B, CH, T, H, W = 4, 512, 8, 14, 14
N = T * H * W            # 1568
D = 64
NHEADS = 8
N_CORES = 8
HPC = 4
BLK = 128
NBLK = (N + BLK - 1) // BLK    # 13
M_LAST = N - (NBLK - 1) * BLK  # 32
NMAIN = 1536
NTAIL = N - NMAIN              # 32
QW = [512, 512, 512, NTAIL]

_BF16 = mybir.dt.bfloat16
_F32 = mybir.dt.float32

LAST_RESULT = None
_NC_CACHE = None


def _build_nc(nrep: int = 1, variant: str = "full"):
    from contextlib import ExitStack

    nc = bacc.Bacc("TRN2")
    q_dram = nc.dram_tensor("q", [HPC, D, N], _BF16, kind="ExternalInput").ap()
    qn_dram = nc.dram_tensor(
        "qn", [HPC, BLK, NBLK * D], _BF16, kind="ExternalInput"
    ).ap()
    o_dram = nc.dram_tensor("o", [HPC, D, N], _BF16, kind="ExternalOutput").ap()

    with tile.TileContext(nc) as tc:
        with (
            tc.tile_pool(name="ps", bufs=1, space="PSUM") as ps,
            tc.tile_pool(name="sb", bufs=1) as sb,
            tc.tile_pool(name="epool", bufs=1) as epool,
            tc.tile_pool(name="small", bufs=1) as small,
            tc.tile_pool(name="scr", bufs=4, space="DRAM") as scr,
            ExitStack() as rep_ctx,
        ):
            if nrep > 1:
                rep_ctx.enter_context(tc.For_i(0, nrep, 1))

            def make_state(h):
                """Load qT twice (rows 0-63 and 64-127) + qn; init rowsum."""
                qTp = sb.tile([2 * D, N], _BF16, tag="qT", bufs=2,
                              name=f"qT_{h}")
                nc.sync.dma_start(out=qTp[0:D, :], in_=q_dram[h])
                nc.sync.dma_start(out=qTp[D : 2 * D, :], in_=q_dram[h])
                qn = sb.tile([BLK, NBLK * D], _BF16, tag="qn", bufs=4,
                             name=f"qn_{h}")
                nc.sync.dma_start(out=qn, in_=qn_dram[h])
                rs = small.tile([BLK, NBLK], _F32, tag="rs", bufs=4,
                                name=f"rs_{h}")
                nc.vector.memset(rs, 1.0)
                return {"qT": qTp, "qn": qn, "rs": rs, "e": []}

            def s_block(h, st, k):
                """S matmuls + exp for block k; row group alternates with
                k parity so consecutive blocks run concurrently on PE."""
                mk = BLK if k < NBLK - 1 else M_LAST
                rows = slice(0, D) if k % 2 == 0 else slice(D, 2 * D)
                s_ps = ps.tile([BLK, NMAIN], _F32, tag="s", bufs=2,
                               name=f"s_{h}_{k}")
                lhsT = st["qT"][rows, k * BLK : k * BLK + mk]
                for c in range(3):
                    nc.tensor.matmul(
                        s_ps[0:mk, c * 512 : (c + 1) * 512], lhsT,
                        st["qT"][rows, c * 512 : (c + 1) * 512],
                        start=True, stop=True,
                    )
                ek = epool.tile([BLK, NMAIN], _BF16, tag="e", bufs=40,
                                name=f"e_{h}_{k}")
                if variant == "tinyact":
                    nc.scalar.activation(
                        ek[0:1, 0:64], s_ps[0:1, 0:64],
                        mybir.ActivationFunctionType.Exp, scale=1.0 / 64.0,
                    )
                elif variant == "dvers" and h < 3:
                    nc.scalar.activation(
                        ek[0:mk, :], s_ps[0:mk, :],
                        mybir.ActivationFunctionType.Exp,
                        scale=1.0 / 64.0,
                    )
                    nc.vector.tensor_reduce(
                        out=st["rs"][0:mk, k : k + 1],
                        in_=ek[0:mk, :],
                        axis=mybir.AxisListType.X,
                        op=mybir.AluOpType.add,
                    )
                else:
                    nc.scalar.activation(
                        ek[0:mk, :], s_ps[0:mk, :],
                        mybir.ActivationFunctionType.Exp,
                        scale=1.0 / 64.0,
                        accum_out=st["rs"][0:mk, k : k + 1],
                    )
                st["e"].append(ek)

            def s_tail_early(h, st):
                """32 trailing S columns for all 13 blocks.  Emitted at
                phase START (independent of the main blocks) so the
                rowsum chain never extends past the last main exp."""
                st_ps = ps.tile([BLK, NBLK * NTAIL], _F32, tag="st", bufs=1,
                                name=f"st_{h}")
                nc.vector.memset(st_ps, 0.0)  # k=12 rows 32:128 -> no inf
                for k in range(NBLK):
                    mk = BLK if k < NBLK - 1 else M_LAST
                    nc.tensor.matmul(
                        st_ps[0:mk, k * NTAIL : (k + 1) * NTAIL],
                        st["qT"][0:D, k * BLK : k * BLK + mk],
                        st["qT"][0:D, NMAIN:N],
                        start=True, stop=True,
                    )
                etail = sb.tile([BLK, NBLK * NTAIL], _BF16, tag="et", bufs=4,
                                name=f"et_{h}")
                nc.scalar.activation(
                    etail, st_ps, mybir.ActivationFunctionType.Exp,
                    scale=1.0 / 64.0,
                )
                tails = small.tile([BLK, NBLK], _F32, tag="ts", bufs=2,
                                   name=f"ts_{h}")
                nc.vector.tensor_reduce(
                    out=tails,
                    in_=etail.rearrange("p (k t) -> p k t", t=NTAIL),
                    axis=mybir.AxisListType.X,
                    op=mybir.AluOpType.add,
                )
                st["et"] = etail
                st["ts"] = tails

            def s_tail_late(h, st):
                nc.vector.tensor_add(st["rs"], st["rs"], st["ts"])

            def normalizer(h, st, pair_st, half):
                """Write 1/rowsum broadcast into half of the pair R tile."""
                recip = small.tile([BLK, 32], _F32, tag="recip", bufs=2,
                                   name=f"rc_{h}")
                nc.vector.reciprocal(recip[:, 0:NBLK], st["rs"])
                rt = small.tile([BLK, 32], _F32, tag="rt", bufs=2,
                                name=f"rt_{h}")
                nc.vector.transpose(rt, recip)
                scratch = scr.tile([NBLK * BLK], _F32, tag="v",
                                   name=f"scr_{h}")
                for a in range(4):
                    # scratch[128k + 32a + i] = recip[32a+i, k]
                    nc.sync.dma_start(
                        out=bass.AP(
                            tensor=scratch.tensor,
                            offset=scratch.offset + 32 * a,
                            ap=[[BLK, NBLK], [1, 32]],
                        ),
                        in_=rt[32 * a : 32 * a + NBLK, :],
                    )
                nc.gpsimd.dma_start(
                    out=pair_st["R"][half * D : (half + 1) * D, :],
                    in_=bass.AP(
                        tensor=scratch.tensor,
                        offset=scratch.offset,
                        ap=[[0, D], [1, N]],
                    ),
                )

            def out_phase(h0, sts, pair_st):
                """Col-tiled out matmuls for heads (h0, h0+1): mm/norm/store
                emit-callables; each quarter's accumulator comes from the
                psum tag given at mm-emission time ("o", or "s" when the
                S banks are free during the drain)."""
                op_tiles = {}

                def mm(qtr, k, tag="o"):
                    def emit():
                        if qtr not in op_tiles:
                            shape = [BLK, 512] if tag == "o" else [BLK, NMAIN]
                            op_tiles[qtr] = ps.tile(
                                shape, _F32, tag=tag, bufs=1 if tag == "o"
                                else 2, name=f"o_{h0}_{qtr}",
                            )
                        op = op_tiles[qtr]
                        mk = BLK if k < NBLK - 1 else M_LAST
                        base, width = qtr * 512, QW[qtr]
                        for j in range(2):
                            st = sts[j]
                            rhs = (
                                st["e"][k][0:mk, base : base + width]
                                if qtr < 3
                                else st["et"][0:mk,
                                              k * NTAIL : (k + 1) * NTAIL]
                            )
                            nc.tensor.matmul(
                                op[j * D : (j + 1) * D, 0:width],
                                st["qn"][0:mk, k * D : (k + 1) * D], rhs,
                                start=(k == 0), stop=(k == NBLK - 1),
                            )
                    return emit

                def norm(qtr):
                    def emit():
                        op = op_tiles.pop(qtr)
                        base, width = qtr * 512, QW[qtr]
                        nc.vector.tensor_mul(
                            pair_st["outT"][:, base : base + width],
                            op[:, 0:width],
                            pair_st["R"][:, base : base + width],
                        )
                    return emit

                def store():
                    for j in range(2):
                        nc.sync.dma_start(
                            out=o_dram[h0 + j],
                            in_=pair_st["outT"][j * D : (j + 1) * D, :],
                        )

                return mm, norm, store

            def out_chunks(h0, sts, pair_st):
                mm, norm, store = out_phase(h0, sts, pair_st)
                for qtr in range(4):
                    for k in range(NBLK):
                        yield mm(qtr, k)
                    yield norm(qtr)
                yield store

            def make_pair_state(h0):
                pst = {
                    "R": sb.tile([2 * D, N], _BF16, tag="R", bufs=2,
                                 name=f"R_{h0}"),
                    "outT": sb.tile([2 * D, N], _BF16, tag="outT", bufs=2,
                                    name=f"outT_{h0}"),
                }
                if variant == "tinyact":
                    nc.vector.memset(pst["R"], 1.0)
                return pst

            def s_phase(h, pair_st, half, interleave=(), per_k=None):
                """Emit S(h); interleave out-chunk callables between blocks.
                per_k(k) emits extra chunks right after block k."""
                st = make_state(h)
                if variant != "tinyact":
                    s_tail_early(h, st)
                else:
                    st["et"] = sb.tile([BLK, NBLK * NTAIL], _BF16, tag="et",
                                       bufs=4, name=f"et_{h}")
                    nc.vector.memset(st["et"], 0.0)
                chunks = list(interleave)
                # Emit blocks in adjacent PAIRS so their row-tiled matmul
                # trios stay back-to-back in the PE queue (out-chunks in
                # between would break the array row-group pairing and
                # serialize the S matmuls at the solo rate).
                ngroups = (NBLK + 1) // 2
                bounds = [len(chunks) * g // ngroups for g in range(ngroups + 1)]
                for g in range(ngroups):
                    pair_ks = range(2 * g, min(2 * g + 2, NBLK))
                    for k in pair_ks:
                        s_block(h, st, k)
                    for k in pair_ks:
                        if per_k is not None:
                            per_k(k, st)
                    for c in chunks[bounds[g] : bounds[g + 1]]:
                        c()
                if variant != "tinyact":
                    s_tail_late(h, st)
                    normalizer(h, st, pair_st, half)
                return st

            # ---- schedule ----------------------------------------------
            pair01 = make_pair_state(0)
            pair23 = make_pair_state(2)
            st0 = s_phase(0, pair01, 0)
            st1 = s_phase(1, pair01, 1)
            # OUT(0,1): all chunks inside S2's ACT window
            out01 = list(out_chunks(0, [st0, st1], pair01))
            st2 = s_phase(2, pair23, 0, interleave=out01)
            # OUT(2,3): quarter 0 accumulates during S3 right behind each
            # exp; the rest drains from the freed S banks afterwards.
            sts23 = [st2, None]
            mm23, norm23, store23 = out_phase(2, sts23, pair23)

            def per_k3(k, st):
                sts23[1] = st
                mm23(0, k, tag="o")()

            st3 = s_phase(3, pair23, 1, per_k=per_k3)
            for k in range(NBLK):
                mm23(1, k, tag="s")()
            norm23(0)()
            for k in range(NBLK):
                mm23(2, k, tag="s")()
            norm23(1)()
            for k in range(NBLK):
                mm23(3, k, tag="o")()
            norm23(2)()
            norm23(3)()
            store23()

    nc.compile()
    return nc


def _prep_inputs(x: np.ndarray) -> list:
    xr = np.asarray(x).reshape(B, D, NHEADS, N)
    in_maps = []
    for c in range(N_CORES):
        b, h0 = c // 2, HPC * (c % 2)
        q_t = np.ascontiguousarray(
            xr[b, :, h0 : h0 + HPC, :].transpose(1, 0, 2)
        ).astype(ml_dtypes.bfloat16)  # [HPC, D, N]
        tok = q_t.transpose(0, 2, 1)  # [HPC, N, D]
        qn = np.zeros((HPC, BLK, NBLK * D), ml_dtypes.bfloat16)
        main = tok[:, : (NBLK - 1) * BLK].reshape(HPC, NBLK - 1, BLK, D)
        qn[:, :, : (NBLK - 1) * D] = (
            main.transpose(0, 2, 1, 3).reshape(HPC, BLK, (NBLK - 1) * D)
        )
        qn[:, :M_LAST, (NBLK - 1) * D :] = tok[:, (NBLK - 1) * BLK :]
        in_maps.append({"q": q_t, "qn": qn})
    return in_maps


def kernel(x: np.ndarray) -> np.ndarray:
    global LAST_RESULT, _NC_CACHE
    assert x.shape == (B, CH, T, H, W) and x.dtype == np.float32
    if _NC_CACHE is None:
        _NC_CACHE = _build_nc()
    nc = _NC_CACHE

    in_maps = _prep_inputs(x)
    last_exc = None
    for attempt in range(3):
        try:
            LAST_RESULT = bass_utils.run_bass_kernel_spmd(
                nc, in_maps, core_ids=list(range(N_CORES))
            )
            break
        except Exception as e:  # noqa: BLE001
            last_exc = e
            import time as _time

            _time.sleep(2.0 + 3.0 * attempt)
    else:
        raise last_exc

    full = np.empty((B, D, NHEADS, N), np.float32)
    for c in range(N_CORES):
        b, h0 = c // 2, HPC * (c % 2)
        o = LAST_RESULT.results[c]["o"]  # [HPC, D, N] bf16
        full[b, :, h0 : h0 + HPC, :] = o.astype(np.float32).transpose(1, 0, 2)
    return full.reshape(B, CH, T, H, W)
